# revision 1
# baseline (speedup 1.0000x reference)
"""Trainium2 Bass kernel: Backprojection3DConsistencyLoss (8-core SPMD).

Contract: kernel(**inputs) takes the FULL unsharded inputs of the reference
(pred_frontal/pred_lateral [2,1,128,128] f32, source/target geometry, the
ground-truth volume [128,128,128] f32, A_inv [3,3], t_inv [3]) and returns the
FULL scalar loss, computing the heavy work on 8 NeuronCores.

Algorithm (data-parallel over rays, per the sharding hint):
  Host prep: each ray's voxel-space trajectory is affine in the sample index,
    voxel(s) = M + G*s (s = 0..511).  M, G are computed per ray; rays with
    mask <= 0.5 are dropped; surviving rays are dealt round-robin to the 8
    cores and padded to groups of 128 with poison rays that can never
    validate.
  Device, per (batch, view) volume and per slice k along the view's dominant
    axis: the unique candidate sample is s* = round((k - M_a)/G_a) (unique
    because |G_a| ~ 2.6 voxels/sample).  Validation (slice hit, y-bounds,
    active) is folded into a single f32 "code" value per ray; one-hot vectors
    of the two in-slice coordinates are built by iota-compare and the
    TensorEngine accumulates count[m,n] = sum_rays oh_m*oh_n in PSUM.
    count > 0.5 is the 0/1 occupancy slice (bf16).
  ReduceScatter(max) over the 8 cores combines partial volumes so core c owns
    z-slices [16c,16c+16) of all 4 volumes; each core evaluates the BCE on its
    shard with the exact quadratic  q0 + q1*s + q2*s^2 + gt*s  (s = volF+volL
    in {0,1,2}; uses log(sigmoid(s)) - log(1-sigmoid(s)) = s), reduced to a
    [128,1] per-partition partial that the host sums and scales.

If the input geometry ever violates the unique-candidate assumption (it holds
for this module's detector geometry), a faithful f32 numpy fallback computes
the same result on host.
"""

import math
import sys

import numpy as np

for _p in ("/opt/trn_rl_repo",):
    if _p not in sys.path:
        sys.path.insert(0, _p)

import concourse.bacc as bacc  # noqa: E402
import concourse.mybir as mybir  # noqa: E402
import concourse.tile as tile  # noqa: E402
from concourse.bass_utils import run_bass_kernel_spmd  # noqa: E402

N_CORES = 8
V = 128          # volume side
S = 512          # samples per ray
SB = 8           # slices per setup block
NQ = 7           # per-ray quantities: Ma, negInvGa, Ga, Mm, Gm, Mn, Gn
MAGIC = 12582912.0  # 1.5*2^23: (x+MAGIC)-MAGIC == f32 round-to-nearest-even
F32 = mybir.dt.float32
BF16 = mybir.dt.bfloat16
ALU = mybir.AluOpType

# BCE quadratic: cell loss = q0 + q1*s + q2*s^2 + gt*s, exact for s in {0,1,2}
_B0 = math.log(0.5)
_B1 = -math.log1p(math.e)
_B2 = -2.0 - math.log1p(math.exp(-2.0))
Q0 = _B0
Q1 = (-3.0 * _B0 + 4.0 * _B1 - _B2) / 2.0
Q2 = (_B0 - 2.0 * _B1 + _B2) / 2.0

_PROGRAM_CACHE: dict = {}


class _GeometryFallback(Exception):
    pass


def _build_program(NG: int):
    if NG in _PROGRAM_CACHE:
        return _PROGRAM_CACHE[NG]

    nc = bacc.Bacc("TRN2", target_bir_lowering=False, debug=False,
                   num_devices=N_CORES)
    rays = nc.declare_dram_parameter("rays", [128, 4, NQ, NG], F32,
                                     isOutput=False)
    gt_p = nc.declare_dram_parameter("gt", [128, 2048], F32, isOutput=False)
    out_vec = nc.declare_dram_parameter("out_vec", [128, 1], F32,
                                        isOutput=True)

    # partial volumes, chunk-major so ReduceScatter hands core c z-slices
    # [16c, 16c+16) of every volume: [chunk, vol, z%16, y, x]
    vols = nc.dram_tensor("vols", [N_CORES, 4, 16, V, V], BF16)
    shard_rs = nc.dram_tensor("shard_rs", [4, 128, 2048], BF16)

    with tile.TileContext(nc) as tc:
        with (
            tc.tile_pool(name="const", bufs=1) as constp,
            tc.tile_pool(name="work", bufs=3) as work,
            tc.tile_pool(name="oh", bufs=6) as ohp,
            tc.tile_pool(name="psum", bufs=4, space="PSUM") as psump,
            tc.tile_pool(name="slice", bufs=6) as slicep,
            tc.tile_pool(name="lat", bufs=2) as latp,
            tc.tile_pool(name="bce", bufs=2) as bcep,
        ):
            iota_i = constp.tile([128, 128], mybir.dt.int32)
            nc.gpsimd.iota(iota_i[:], pattern=[[1, 128]], base=0,
                           channel_multiplier=0)
            iota_b = constp.tile([128, 128], BF16)
            nc.vector.tensor_copy(iota_b[:], iota_i[:])

            kvec_i = constp.tile([128, V, NG], mybir.dt.int32)
            nc.gpsimd.iota(kvec_i[:], pattern=[[1, V], [0, NG]], base=0,
                           channel_multiplier=0)
            kvec = constp.tile([128, V, NG], F32)
            nc.vector.tensor_copy(kvec[:], kvec_i[:])

            rays_sb = constp.tile([128, 4, NQ, NG], F32)
            nc.sync.dma_start(rays_sb[:], rays.ap())

            def bc(ap):  # [128, NG] -> [128, SB, NG] broadcast
                return ap.unsqueeze(1).broadcast_to([128, SB, NG])

            for v in range(4):
                is_lat = (v % 2 == 1)
                q = lambda i: rays_sb[:, v, i, :]  # noqa: E731
                if is_lat:
                    # full z-major volume staged in SBUF: [z, y, x]
                    latvol = latp.tile([128, V, V], BF16, tag="latvol")
                for kb in range(V // SB):
                    ks = kvec[:, kb * SB:(kb + 1) * SB, :]
                    sr = work.tile([128, SB, NG], F32, tag="sr")
                    nc.vector.tensor_tensor(sr[:], bc(q(0)), ks, ALU.subtract)
                    nc.vector.tensor_tensor(sr[:], sr[:], bc(q(1)), ALU.mult)
                    nc.vector.tensor_scalar(sr[:], sr[:], MAGIC, MAGIC,
                                            ALU.add, ALU.subtract)
                    nc.vector.tensor_scalar(sr[:], sr[:], 0.0, 511.0,
                                            ALU.max, ALU.min)
                    zf = work.tile([128, SB, NG], F32, tag="zf")
                    nc.vector.tensor_tensor(zf[:], bc(q(2)), sr[:], ALU.mult)
                    nc.vector.tensor_tensor(zf[:], zf[:], bc(q(0)), ALU.add)
                    nc.vector.tensor_scalar(zf[:], zf[:], MAGIC, MAGIC,
                                            ALU.add, ALU.subtract)
                    nc.vector.tensor_tensor(zf[:], zf[:], ks, ALU.subtract)
                    nc.vector.tensor_scalar(zf[:], zf[:], 1000.0, None,
                                            ALU.mult)
                    ym = work.tile([128, SB, NG], F32, tag="ym")
                    nc.vector.tensor_tensor(ym[:], bc(q(4)), sr[:], ALU.mult)
                    nc.vector.tensor_tensor(ym[:], ym[:], bc(q(3)), ALU.add)
                    nc.vector.tensor_scalar(ym[:], ym[:], MAGIC, MAGIC,
                                            ALU.add, ALU.subtract)
                    nc.vector.tensor_scalar(ym[:], ym[:], -2.0, 129.0,
                                            ALU.max, ALU.min)
                    nc.vector.tensor_tensor(ym[:], ym[:], zf[:], ALU.add)
                    xn = work.tile([128, SB, NG], F32, tag="xn")
                    nc.vector.tensor_tensor(xn[:], bc(q(6)), sr[:], ALU.mult)
                    nc.vector.tensor_tensor(xn[:], xn[:], bc(q(5)), ALU.add)
                    nc.vector.tensor_scalar(xn[:], xn[:], MAGIC, MAGIC,
                                            ALU.add, ALU.subtract)
                    for j in range(SB):
                        k = kb * SB + j
                        ps = psump.tile([128, 128], F32)
                        for g in range(NG):
                            ohm = ohp.tile([128, 128], BF16, tag="ohm")
                            nc.vector.tensor_scalar(
                                ohm[:], iota_b[:], ym[:, j, g:g + 1], None,
                                ALU.is_equal)
                            ohn = ohp.tile([128, 128], BF16, tag="ohn")
                            nc.vector.tensor_scalar(
                                ohn[:], iota_b[:], xn[:, j, g:g + 1], None,
                                ALU.is_equal)
                            nc.tensor.matmul(ps[:], lhsT=ohm[:], rhs=ohn[:],
                                             start=(g == 0), stop=(g == NG - 1))
                        # occupancy = Sign(count) in {0,1}; runs on the
                        # otherwise-idle ScalarEngine
                        if is_lat:
                            nc.scalar.sign(latvol[:, :, k], ps[:])
                        else:
                            sl = slicep.tile([128, 128], BF16, tag="sl")
                            nc.scalar.sign(sl[:], ps[:])
                            nc.sync.dma_start(vols[k // 16, v, k % 16], sl[:])
                if is_lat:
                    nc.sync.dma_start(vols[:, v], latvol[:])

            nc.gpsimd.collective_compute(
                "ReduceScatter", ALU.max,
                replica_groups=[list(range(N_CORES))],
                ins=[vols.ap()], outs=[shard_rs.ap()],
            )

            gtt = bcep.tile([128, 2048], F32, tag="gt")
            nc.sync.dma_start(gtt[:], gt_p.ap())
            accs = []
            for b in range(2):
                Ft = bcep.tile([128, 2048], BF16, tag="F")
                nc.sync.dma_start(Ft[:], shard_rs[2 * b + 0])
                Lt = bcep.tile([128, 2048], BF16, tag="L")
                nc.sync.dma_start(Lt[:], shard_rs[2 * b + 1])
                s = bcep.tile([128, 2048], F32, tag="s")
                nc.vector.tensor_tensor(s[:], Ft[:], Lt[:], ALU.add)
                t1 = bcep.tile([128, 2048], F32, tag="t1")
                nc.vector.tensor_scalar(t1[:], s[:], float(Q2), float(Q1),
                                        ALU.mult, ALU.add)
                u = bcep.tile([128, 2048], F32, tag="u")
                nc.vector.tensor_tensor(u[:], t1[:], s[:], ALU.mult)
                w = bcep.tile([128, 2048], F32, tag="w")
                nc.vector.tensor_tensor(w[:], gtt[:], s[:], ALU.mult)
                cell = bcep.tile([128, 2048], F32, tag="cell")
                acc = bcep.tile([128, 1], F32, tag=f"acc{b}")
                nc.vector.scalar_tensor_tensor(
                    out=cell[:], in0=u[:], scalar=float(Q0), in1=w[:],
                    op0=ALU.add, op1=ALU.add, accum_out=acc[:])
                accs.append(acc)
            total = bcep.tile([128, 1], F32, tag="total")
            nc.vector.tensor_tensor(total[:], accs[0][:], accs[1][:], ALU.add)
            nc.sync.dma_start(out_vec.ap(), total[:])

    nc.compile()
    _PROGRAM_CACHE[NG] = nc
    return nc


def _host_prep(inputs):
    f32 = np.float32
    pf = np.asarray(inputs["pred_frontal"], dtype=f32)
    pl = np.asarray(inputs["pred_lateral"], dtype=f32)
    srcF = np.asarray(inputs["source_F"], dtype=f32)[0]
    tgtF = np.asarray(inputs["target_F"], dtype=f32)[0]
    srcL = np.asarray(inputs["source_L"], dtype=f32)[0]
    tgtL = np.asarray(inputs["target_L"], dtype=f32)[0]
    A_inv = np.asarray(inputs["A_inv"], dtype=f32)
    t_inv = np.asarray(inputs["t_inv"], dtype=f32)
    gt = np.asarray(inputs["vol_gt_3d"], dtype=f32)
    B = pf.shape[0]
    if B != 2 or gt.shape != (V, V, V) or pf.shape[2] != V:
        raise _GeometryFallback(f"unexpected shapes B={B}")

    def ray_params(src, tgt):
        det = tgt.reshape(-1, 3).astype(f32)
        rd = (det - src[None, :]).astype(f32)
        rl = np.sqrt((rd.astype(np.float64) ** 2).sum(1))[:, None]
        rdn = rd.astype(np.float64) / (rl + 1e-8)
        Dv = rdn * (rl * 2.5) / 511.0
        A64 = A_inv.astype(np.float64)
        M = src.astype(np.float64) @ A64.T + t_inv.astype(np.float64)
        M = np.broadcast_to(M, Dv.shape)
        G = Dv @ A64.T
        return M, G

    MF, GF = ray_params(srcF, tgtF)
    ML, GL = ray_params(srcL, tgtL)
    axF = int(np.argmax(np.abs(GF).mean(0)))
    axL = int(np.argmax(np.abs(GL).mean(0)))

    vols_meta = []
    for b in range(B):
        vols_meta.append((MF, GF, axF,
                          np.flatnonzero(pf[b, 0].reshape(-1) > 0.5)))
        vols_meta.append((ML, GL, axL,
                          np.flatnonzero(pl[b, 0].reshape(-1) > 0.5)))

    for M, G, ax, idx in vols_meta:
        if ax not in (0, 2):
            raise _GeometryFallback(f"axis {ax} layout unsupported")
        if len(idx) and np.abs(G[idx, ax]).min() < 1.6:
            raise _GeometryFallback("slice-inversion uniqueness violated")

    NG = 1
    percore = []
    for M, G, ax, idx in vols_meta:
        counts = [len(idx[c::N_CORES]) for c in range(N_CORES)]
        if max(counts):
            NG = max(NG, max(-(-c // 128) for c in counts))
        percore.append([idx[c::N_CORES] for c in range(N_CORES)])

    gtz = np.ascontiguousarray(gt.transpose(2, 1, 0))  # z-major [z][y][x]
    in_maps = []
    for c in range(N_CORES):
        arr = np.zeros((128, 4, NQ, NG), dtype=f32)
        arr[:, :, 3, :] = 1e9  # poison: code_m clamps to 129, never valid
        for v, (M, G, ax, _) in enumerate(vols_meta):
            idx = percore[v][c]
            n = len(idx)
            if n == 0:
                continue
            # m = PSUM partition coord, n = PSUM free coord:
            # frontal (ax=2, z-slices): m=y(1), n=x(0)
            # lateral (ax=0, x-slices): m=z(2), n=y(1)
            m_ax, n_ax = (1, 0) if ax == 2 else (2, 1)
            g_ = np.arange(n) // 128
            p_ = np.arange(n) % 128
            vals = [M[idx, ax], -1.0 / G[idx, ax], G[idx, ax],
                    M[idx, m_ax], G[idx, m_ax], M[idx, n_ax], G[idx, n_ax]]
            for qi, va in enumerate(vals):
                arr[p_, v, qi, g_] = va.astype(f32)
        gshard = gtz[16 * c:16 * (c + 1)].reshape(128, 2048)
        in_maps.append({"rays": arr, "gt": np.ascontiguousarray(gshard)})
    return in_maps, NG


def _reference_fallback(inputs):
    """Faithful f32 numpy replica of the jax reference (safety net)."""
    f32 = np.float32
    pf = np.asarray(inputs["pred_frontal"], dtype=f32)
    pl = np.asarray(inputs["pred_lateral"], dtype=f32)
    srcF = np.asarray(inputs["source_F"], dtype=f32)[0]
    tgtF = np.asarray(inputs["target_F"], dtype=f32)[0]
    srcL = np.asarray(inputs["source_L"], dtype=f32)[0]
    tgtL = np.asarray(inputs["target_L"], dtype=f32)[0]
    A_inv = np.asarray(inputs["A_inv"], dtype=f32)
    t_inv = np.asarray(inputs["t_inv"], dtype=f32)
    gt = np.asarray(inputs["vol_gt_3d"], dtype=f32)

    def backproject(mask2d, src, tgt):
        active = (mask2d > 0.5).reshape(-1)
        det = tgt.reshape(-1, 3).astype(f32)
        rd = (det - src[None, :]).astype(f32)
        rl = np.sqrt((rd * rd).sum(1, dtype=f32)).astype(f32)[:, None]
        rdn = (rd / (rl + f32(1e-8))).astype(f32)
        tv = (np.arange(S, dtype=f32) * (f32(1.0) / f32(511.0)))
        ts = (tv[None, :, None] * (rl[:, None, :] * f32(2.5))).astype(f32)
        world = (src[None, None, :] + rdn[:, None, :] * ts).astype(f32)
        vox_f = (world @ A_inv.T + t_inv).astype(f32)
        vox = np.rint(vox_f).astype(np.int64)
        ok = (active[:, None]
              & (vox[..., 0] >= 0) & (vox[..., 0] < V)
              & (vox[..., 1] >= 0) & (vox[..., 1] < V)
              & (vox[..., 2] >= 0) & (vox[..., 2] < V))
        vi = np.clip(vox, 0, V - 1)
        vol = np.zeros((V, V, V), dtype=f32)
        flat = (vi[..., 0] * V + vi[..., 1]) * V + vi[..., 2]
        vol.reshape(-1)[flat[ok]] = 1.0
        return vol

    total = 0.0
    B = pf.shape[0]
    for b in range(B):
        vF = backproject(pf[b, 0], srcF, tgtF)
        vL = backproject(pl[b, 0], srcL, tgtL)
        sv = (vF + vL).astype(np.float64)
        p = 1.0 / (1.0 + np.exp(-sv))
        total += -(gt * np.log(p) + (1.0 - gt) * np.log1p(-p)).mean()
    return np.float32(total / B)


def kernel(**inputs) -> np.ndarray:
    try:
        in_maps, NG = _host_prep(inputs)
    except _GeometryFallback:
        return _reference_fallback(inputs)
    nc = _build_program(NG)
    res = run_bass_kernel_spmd(nc, in_maps, list(range(N_CORES)))
    total = sum(float(r["out_vec"].sum()) for r in res.results)
    return np.float32(-total / (2.0 * V * V * V))



# revision 6
# speedup vs baseline: 3.6189x; 3.6189x over previous
"""Trainium2 Bass kernel: Backprojection3DConsistencyLoss (8-core SPMD).

Contract: kernel(**inputs) takes the FULL unsharded inputs of the reference
(pred_frontal/pred_lateral [2,1,128,128] f32, source/target geometry, the
ground-truth volume [128,128,128] f32, A_inv [3,3], t_inv [3]) and returns the
FULL scalar loss, computing the heavy work on 8 NeuronCores.

Algorithm (separable per-slice reconstruction; no collectives):
  For this module's geometry the detector plane is constant along the scan
  axis, so every ray shares the same scan-axis sample sequence
  z_s = src_z + dz*(2.5 s/511) (the ray-length normalization cancels exactly).
  Slice k therefore has a unique shared sample s_k (or none), and within the
  slice the hit voxel is a separable affine map of the detector indices:
  y = rint(sy + (j-sy)*a_k), x = rint(sx + (i-sx)*a_k) with a_k = 2.5 s_k/511.
  With one-hot matrices A_k[p, v] = [map_k(p) == v] the 0/1 slice image is
  sign(A_k^T . mask^T-ish . A_k) — two small matmuls on the TensorEngine.

  Sharding: core c owns z-slices [16c, 16c+16) of all four volumes.
  Frontal volumes are built slice-by-slice (z = slice axis); lateral volumes
  (x = slice axis) are built as the 16-column z-band of every x-slice, via a
  skinny pair of matmuls per non-empty x-slice.  All per-core variation is
  carried by host-computed bf16 lookup tables, so one SPMD program serves all
  8 cores and no ReduceScatter is needed.  Each core evaluates its BCE shard
  with the exact quadratic q0 + q1*s + q2*s^2 + gt*s (s in {0,1,2}) reduced
  on-device to a few per-partition sums; the host combines them.

If the input geometry violates the separability assumptions (it holds for
this module's detector geometry; margins are checked in f64), a faithful f32
numpy fallback computes the same result on host.
"""

import math
import sys

import numpy as np

for _p in ("/opt/trn_rl_repo",):
    if _p not in sys.path:
        sys.path.insert(0, _p)

import ml_dtypes  # noqa: E402

import concourse.bacc as bacc  # noqa: E402
import concourse.mybir as mybir  # noqa: E402
import concourse.tile as tile  # noqa: E402
from concourse.bass_utils import run_bass_kernel_spmd  # noqa: E402

N_CORES = 8
V = 128          # volume side
S = 512          # samples per ray
ZW = V // N_CORES  # z-slices per core (16)
POISON = 200.0   # one-hot compare value that never matches a voxel coordinate
F32 = mybir.dt.float32
BF16 = mybir.dt.bfloat16
I32 = mybir.dt.int32
ALU = mybir.AluOpType

# BCE quadratic: cell loss = q0 + q1*s + q2*s^2 + gt*s, exact for s in {0,1,2}
_B0 = math.log(0.5)
_B1 = -math.log1p(math.e)
_B2 = -2.0 - math.log1p(math.exp(-2.0))
Q0 = _B0
Q1 = (-3.0 * _B0 + 4.0 * _B1 - _B2) / 2.0
Q2 = (_B0 - 2.0 * _B1 + _B2) / 2.0

# packed bf16 table layout (columns of the [128, 800] "tabs" input)
_C_VXF = 0      # [128, 16]  frontal x map for the core's 16 z-slices
_C_VYF = 16     # [128, 16]  frontal y map
_C_VYL = 32     # [128, 128] lateral y map (all x-slices)
_C_VZL = 160    # [128, 128] lateral z map, shifted by -16c
_C_MF = 288     # [128, 2*128] frontal masks (i-partition, j-free)
_C_MLT = 544    # [128, 2*128] lateral masks transposed (j-partition, i-free)
_TABS_W = 800

_PROGRAM_CACHE: dict = {}


class _GeometryFallback(Exception):
    pass


def _build_program(nl_ks: tuple):
    key = nl_ks
    if key in _PROGRAM_CACHE:
        return _PROGRAM_CACHE[key]

    nc = bacc.Bacc("TRN2", target_bir_lowering=False, debug=False,
                   num_devices=N_CORES)
    tabs = nc.declare_dram_parameter("tabs", [128, _TABS_W], BF16,
                                     isOutput=False)
    gt_p = nc.declare_dram_parameter("gt", [128, ZW * V], F32, isOutput=False)
    out_p = nc.declare_dram_parameter("out_vec", [128, 8], F32, isOutput=True)

    with tile.TileContext(nc) as tc:
        with (
            tc.tile_pool(name="const", bufs=1) as constp,
            tc.tile_pool(name="vsb", bufs=3) as vsbp,
            tc.tile_pool(name="wsb", bufs=2) as wsbp,
            tc.tile_pool(name="bce", bufs=2) as bcep,
            tc.tile_pool(name="psV", bufs=2, space="PSUM") as psVp,
            tc.tile_pool(name="psCF", bufs=2, space="PSUM") as psCFp,
            tc.tile_pool(name="psW", bufs=2, space="PSUM") as psWp,
            tc.tile_pool(name="psC", bufs=2, space="PSUM") as psCp,
        ):
            tabs_sb = constp.tile([128, _TABS_W], BF16)
            nc.sync.dma_start(tabs_sb[:], tabs.ap())
            gt_sb = constp.tile([128, ZW, V], F32)
            nc.sync.dma_start(gt_sb[:], gt_p.ap())

            iota_i = constp.tile([128, V], I32)
            nc.gpsimd.iota(iota_i[:], pattern=[[1, V]], base=0,
                           channel_multiplier=0)
            iota_b = constp.tile([128, V], BF16)
            nc.vector.tensor_copy(iota_b[:], iota_i[:])

            def one_hot(name, val_ap, nk, nv):
                """AT[p, k, v] = (val[p, k] == v), bf16 {0,1}."""
                t = constp.tile([128, nk, nv], BF16, tag=name)
                nc.vector.tensor_tensor(
                    t[:],
                    iota_b[:, 0:nv].unsqueeze(1).broadcast_to([128, nk, nv]),
                    val_ap.unsqueeze(2).broadcast_to([128, nk, nv]),
                    ALU.is_equal)
                return t

            ATxF = one_hot("ATxF", tabs_sb[:, _C_VXF:_C_VXF + ZW], ZW, V)
            ATyF = one_hot("ATyF", tabs_sb[:, _C_VYF:_C_VYF + ZW], ZW, V)
            ATzL = one_hot("ATzL", tabs_sb[:, _C_VZL:_C_VZL + V], V, ZW)
            ATyL = one_hot("ATyL", tabs_sb[:, _C_VYL:_C_VYL + V], V, V)

            def maskF(b):   # [i, j]
                return tabs_sb[:, _C_MF + V * b:_C_MF + V * (b + 1)]

            def maskLT(b):  # [j, i]
                return tabs_sb[:, _C_MLT + V * b:_C_MLT + V * (b + 1)]

            volF = constp.tile([128, 2, ZW, V], BF16)
            volL = constp.tile([128, 2, ZW, V], BF16)
            nc.gpsimd.memset(volL[:], 0.0)

            # ---- frontal: full [y, x] slice per owned z ----
            for kk in range(ZW):
                psV = psVp.tile([128, 2, V], F32)
                for b in range(2):
                    nc.tensor.matmul(psV[:, b, :], lhsT=maskF(b),
                                     rhs=ATxF[:, kk, :], start=True, stop=True)
                vsb = vsbp.tile([128, 2, V], BF16, tag="v")
                nc.vector.tensor_copy(vsb[:], psV[:])
                psC = psCFp.tile([128, 2, V], F32)
                nc.tensor.matmul(psC[:], lhsT=ATyF[:, kk, :], rhs=vsb[:],
                                 start=True, stop=True)
                nc.scalar.sign(volF[:, :, kk, :], psC[:])

            # ---- lateral: [y, z-band] columns of each non-empty x-slice,
            #      batched 8 slices per PSUM bank ----
            LB = 8
            for blk in range(0, len(nl_ks), LB):
                ks = nl_ks[blk:blk + LB]
                psW = psWp.tile([128, LB, 2, ZW], F32)
                for slot, k in enumerate(ks):
                    for b in range(2):
                        nc.tensor.matmul(psW[:, slot, b, :], lhsT=maskLT(b),
                                         rhs=ATzL[:, k, :],
                                         start=True, stop=True)
                wsb = wsbp.tile([128, LB, 2, ZW], BF16, tag="w")
                nc.vector.tensor_copy(wsb[:, 0:len(ks)], psW[:, 0:len(ks)])
                psC2 = psCp.tile([128, LB, 2, ZW], F32)
                for slot, k in enumerate(ks):
                    nc.tensor.matmul(psC2[:, slot], lhsT=ATyL[:, k, :],
                                     rhs=wsb[:, slot], start=True, stop=True)
                for slot, k in enumerate(ks):
                    nc.scalar.sign(volL[:, :, :, k], psC2[:, slot])

            # ---- BCE partial sums: out cols (3b+0, 3b+1, 3b+2) =
            #      sum(s), sum(s==2), sum(gt*s) for batch b ----
            out_sb = constp.tile([128, 8], F32)
            nc.gpsimd.memset(out_sb[:], 0.0)
            for b in range(2):
                s = bcep.tile([128, ZW, V], F32, tag="s")
                nc.vector.scalar_tensor_tensor(
                    out=s[:], in0=volF[:, b], scalar=0.0, in1=volL[:, b],
                    op0=ALU.add, op1=ALU.add,
                    accum_out=out_sb[:, 3 * b:3 * b + 1])
                e2 = bcep.tile([128, ZW, V], F32, tag="e2")
                nc.vector.tensor_scalar(
                    e2[:], s[:], 2.0, 0.0, ALU.is_equal, ALU.add,
                    accum_out=out_sb[:, 3 * b + 1:3 * b + 2])
                gs = bcep.tile([128, ZW, V], F32, tag="gs")
                nc.vector.scalar_tensor_tensor(
                    out=gs[:], in0=gt_sb[:], scalar=1.0, in1=s[:],
                    op0=ALU.mult, op1=ALU.mult,
                    accum_out=out_sb[:, 3 * b + 2:3 * b + 3])
            nc.sync.dma_start(out_p.ap(), out_sb[:])

    nc.compile()
    _PROGRAM_CACHE[key] = nc
    return nc


def _host_prep(inputs):
    """Validate geometry and build per-core bf16 tables.

    Returns (in_maps, nl_ks).  Raises _GeometryFallback when the separability
    assumptions don't hold.
    """
    f32 = np.float32
    pf = np.asarray(inputs["pred_frontal"], dtype=f32)
    pl = np.asarray(inputs["pred_lateral"], dtype=f32)
    srcF = np.asarray(inputs["source_F"], dtype=np.float64)[0]
    tgtF = np.asarray(inputs["target_F"], dtype=np.float64)[0]
    srcL = np.asarray(inputs["source_L"], dtype=np.float64)[0]
    tgtL = np.asarray(inputs["target_L"], dtype=np.float64)[0]
    A_inv = np.asarray(inputs["A_inv"], dtype=np.float64)
    t_inv = np.asarray(inputs["t_inv"], dtype=np.float64)
    gt = np.asarray(inputs["vol_gt_3d"], dtype=f32)
    B = pf.shape[0]
    if B != 2 or gt.shape != (V, V, V) or pf.shape[2:] != (V, V):
        raise _GeometryFallback(f"unexpected shapes B={B}")
    if not np.array_equal(A_inv, np.diag(np.diag(A_inv))):
        raise _GeometryFallback("A_inv not diagonal")
    D = np.diag(A_inv)

    def view_tables(src, tgt, scan_ax, ax_i, ax_j):
        """Per-slice sample index + separable coordinate maps (f64)."""
        # target coordinate along scan axis must be globally constant;
        # along ax_i it may depend only on detector row i, ax_j only on j.
        c = tgt[0, 0, scan_ax]
        if not np.all(tgt[..., scan_ax] == c):
            raise _GeometryFallback("scan axis not constant")
        ti = tgt[:, 0, ax_i]
        if not np.all(tgt[..., ax_i] == ti[:, None]):
            raise _GeometryFallback("ax_i not separable")
        tj = tgt[0, :, ax_j]
        if not np.all(tgt[..., ax_j] == tj[None, :]):
            raise _GeometryFallback("ax_j not separable")

        beta = 2.5 * np.arange(S, dtype=np.float64) / (S - 1.0)
        zeta = (src[scan_ax] + (c - src[scan_ax]) * beta) * D[scan_ax] \
            + t_inv[scan_ax]
        ks = np.rint(zeta).astype(np.int64)
        margin = np.abs(np.abs(zeta - np.rint(zeta)) - 0.5).min()
        if margin < 5e-4:
            raise _GeometryFallback(f"scan margin {margin:.1e}")
        inb = (ks >= 0) & (ks < V)
        if len(np.unique(ks[inb])) != int(inb.sum()):
            raise _GeometryFallback("multiple samples per slice")
        s_for_k = np.full(V, -1, np.int64)
        s_for_k[ks[inb]] = np.arange(S)[inb]

        p = np.arange(V, dtype=np.float64)

        def cmap(tvals, axis):
            """[p, k] voxel coordinate map with POISON for invalid entries."""
            out = np.full((V, V), POISON, dtype=np.float64)
            for k in range(V):
                sk = s_for_k[k]
                if sk < 0:
                    continue
                a = beta[sk]
                w = (src[axis] + (tvals - src[axis]) * a) * D[axis] \
                    + t_inv[axis]
                m = np.abs(np.abs(w - np.rint(w)) - 0.5).min()
                if m < 5e-4:
                    raise _GeometryFallback(f"transverse margin {m:.1e}")
                r = np.rint(w)
                r[(r < 0) | (r >= V)] = POISON
                out[:, k] = r
            return out

        return s_for_k, cmap(ti, ax_i), cmap(tj, ax_j)

    # frontal: scan z(2), i -> vol axis 0 (x), j -> vol axis 1 (y)
    sfF, mapxF, mapyF = view_tables(srcF, tgtF, 2, 0, 1)
    # lateral: scan x(0), i -> vol axis 1 (y), j -> vol axis 2 (z)
    sfL, mapyL, mapzL = view_tables(srcL, tgtL, 0, 1, 2)

    nl_ks = tuple(int(k) for k in range(V) if sfL[k] >= 0)
    if not nl_ks:
        nl_ks = (0,)  # degenerate but keeps the program shape valid

    bf16 = ml_dtypes.bfloat16
    maskF = (pf[:, 0] > 0.5)                       # [b, i, j]
    maskLT = (pl[:, 0] > 0.5).transpose(0, 2, 1)    # [b, j, i]
    gtzyx = np.ascontiguousarray(gt.transpose(1, 2, 0))  # [y][z][x]

    in_maps = []
    for cidx in range(N_CORES):
        z0 = ZW * cidx
        tabs = np.full((128, _TABS_W), POISON, dtype=np.float64)
        tabs[:, _C_VXF:_C_VXF + ZW] = mapxF[:, z0:z0 + ZW]
        tabs[:, _C_VYF:_C_VYF + ZW] = mapyF[:, z0:z0 + ZW]
        tabs[:, _C_VYL:_C_VYL + V] = mapyL
        vz = mapzL.copy()
        ok = vz != POISON
        vz[ok] = vz[ok] - z0
        tabs[:, _C_VZL:_C_VZL + V] = vz
        tabs[:, _C_MF:_C_MF + V] = maskF[0]
        tabs[:, _C_MF + V:_C_MF + 2 * V] = maskF[1]
        tabs[:, _C_MLT:_C_MLT + V] = maskLT[0]
        tabs[:, _C_MLT + V:_C_MLT + 2 * V] = maskLT[1]
        gshard = gtzyx[:, z0:z0 + ZW, :].reshape(128, ZW * V)
        in_maps.append({"tabs": tabs.astype(bf16),
                        "gt": np.ascontiguousarray(gshard)})
    return in_maps, nl_ks


def _combine(results) -> np.ndarray:
    """Host-side reduction of the 8 per-core [128, 8] partial-sum tensors."""
    acc = np.zeros(8, dtype=np.float64)
    for r in results:
        acc += np.asarray(r["out_vec"], dtype=np.float64).sum(axis=0)
    total = 0.0
    for b in range(2):
        ss, se2, sgs = acc[3 * b], acc[3 * b + 1], acc[3 * b + 2]
        total += Q0 * (V ** 3) + (Q1 + Q2) * ss + 2.0 * Q2 * se2 + sgs
    return np.float32(-total / (2.0 * V ** 3))


def _reference_fallback(inputs):
    """Faithful f32 numpy replica of the jax reference (safety net)."""
    f32 = np.float32
    pf = np.asarray(inputs["pred_frontal"], dtype=f32)
    pl = np.asarray(inputs["pred_lateral"], dtype=f32)
    srcF = np.asarray(inputs["source_F"], dtype=f32)[0]
    tgtF = np.asarray(inputs["target_F"], dtype=f32)[0]
    srcL = np.asarray(inputs["source_L"], dtype=f32)[0]
    tgtL = np.asarray(inputs["target_L"], dtype=f32)[0]
    A_inv = np.asarray(inputs["A_inv"], dtype=f32)
    t_inv = np.asarray(inputs["t_inv"], dtype=f32)
    gt = np.asarray(inputs["vol_gt_3d"], dtype=f32)

    def backproject(mask2d, src, tgt):
        active = (mask2d > 0.5).reshape(-1)
        det = tgt.reshape(-1, 3).astype(f32)
        rd = (det - src[None, :]).astype(f32)
        rl = np.sqrt((rd * rd).sum(1, dtype=f32)).astype(f32)[:, None]
        rdn = (rd / (rl + f32(1e-8))).astype(f32)
        tv = (np.arange(S, dtype=f32) * (f32(1.0) / f32(S - 1)))
        ts = (tv[None, :, None] * (rl[:, None, :] * f32(2.5))).astype(f32)
        world = (src[None, None, :] + rdn[:, None, :] * ts).astype(f32)
        vox_f = (world @ A_inv.T + t_inv).astype(f32)
        vox = np.rint(vox_f).astype(np.int64)
        ok = (active[:, None]
              & (vox[..., 0] >= 0) & (vox[..., 0] < V)
              & (vox[..., 1] >= 0) & (vox[..., 1] < V)
              & (vox[..., 2] >= 0) & (vox[..., 2] < V))
        vi = np.clip(vox, 0, V - 1)
        vol = np.zeros((V, V, V), dtype=f32)
        flat = (vi[..., 0] * V + vi[..., 1]) * V + vi[..., 2]
        vol.reshape(-1)[flat[ok]] = 1.0
        return vol

    total = 0.0
    B = pf.shape[0]
    for b in range(B):
        vF = backproject(pf[b, 0], srcF, tgtF)
        vL = backproject(pl[b, 0], srcL, tgtL)
        sv = (vF + vL).astype(np.float64)
        p = 1.0 / (1.0 + np.exp(-sv))
        total += -(gt * np.log(p) + (1.0 - gt) * np.log1p(-p)).mean()
    return np.float32(total / B)


def kernel(**inputs) -> np.ndarray:
    try:
        in_maps, nl_ks = _host_prep(inputs)
    except _GeometryFallback:
        return _reference_fallback(inputs)
    nc = _build_program(nl_ks)
    res = run_bass_kernel_spmd(nc, in_maps, list(range(N_CORES)))
    return _combine(res.results)


# revision 13
# speedup vs baseline: 3.6882x; 1.0192x over previous
"""Trainium2 Bass kernel: Backprojection3DConsistencyLoss (8-core SPMD).

Contract: kernel(**inputs) takes the FULL unsharded inputs of the reference
(pred_frontal/pred_lateral [2,1,128,128] f32, source/target geometry, the
ground-truth volume [128,128,128] f32, A_inv [3,3], t_inv [3]) and returns the
FULL scalar loss, computing the heavy work on 8 NeuronCores.

Algorithm (separable per-slice reconstruction; no collectives):
  For this module's geometry the detector plane is constant along the scan
  axis, so every ray shares the same scan-axis sample sequence
  z_s = src_z + dz*(2.5 s/511) (the ray-length normalization cancels exactly).
  Slice k therefore has a unique shared sample s_k (or none), and within the
  slice the hit voxel is a separable affine map of the detector indices:
  y = rint(sy + (j-sy)*a_k), x = rint(sx + (i-sx)*a_k) with a_k = 2.5 s_k/511.
  With one-hot matrices A_k[p, v] = [map_k(p) == v] the 0/1 slice image is
  sign(A_k^T . mask^T-ish . A_k) — two small matmuls on the TensorEngine.

  Sharding: core c owns z-slices [16c, 16c+16) of all four volumes.
  Frontal volumes are built slice-by-slice (z = slice axis); lateral volumes
  (x = slice axis) are built as the 16-column z-band of every x-slice, via a
  skinny pair of matmuls per non-empty x-slice.  All per-core variation is
  carried by host-computed bf16 lookup tables, so one SPMD program serves all
  8 cores and no ReduceScatter is needed.  Each core evaluates its BCE shard
  with the exact quadratic q0 + q1*s + q2*s^2 + gt*s (s in {0,1,2}) reduced
  on-device to a few per-partition sums; the host combines them.

If the input geometry violates the separability assumptions (it holds for
this module's detector geometry; margins are checked in f64), a faithful f32
numpy fallback computes the same result on host.
"""

import math
import sys

import numpy as np

for _p in ("/opt/trn_rl_repo",):
    if _p not in sys.path:
        sys.path.insert(0, _p)

import ml_dtypes  # noqa: E402

import concourse.bacc as bacc  # noqa: E402
import concourse.mybir as mybir  # noqa: E402
import concourse.tile as tile  # noqa: E402
from concourse.bass_utils import run_bass_kernel_spmd  # noqa: E402

N_CORES = 8
V = 128          # volume side
S = 512          # samples per ray
ZW = V // N_CORES  # z-slices per core (16)
POISON = 200.0   # one-hot compare value that never matches a voxel coordinate
F32 = mybir.dt.float32
BF16 = mybir.dt.bfloat16
I32 = mybir.dt.int32
ALU = mybir.AluOpType

# BCE quadratic: cell loss = q0 + q1*s + q2*s^2 + gt*s, exact for s in {0,1,2}
_B0 = math.log(0.5)
_B1 = -math.log1p(math.e)
_B2 = -2.0 - math.log1p(math.exp(-2.0))
Q0 = _B0
Q1 = (-3.0 * _B0 + 4.0 * _B1 - _B2) / 2.0
Q2 = (_B0 - 2.0 * _B1 + _B2) / 2.0

# packed bf16 table layout (columns of the [128, W] "tabs" input); lateral
# tables carry only the NL non-empty x-slices, in nl_ks order
_C_VXF = 0      # [128, 16]  frontal x map for the core's 16 z-slices
_C_VYF = 16     # [128, 16]  frontal y map
_C_VYL = 32     # [128, NL]  lateral y map
# _C_VZL = 32+NL  [128, NL]  lateral z map, shifted by -16c
# _C_MF  = 32+2NL [128, 2*128] frontal masks (i-partition, j-free)
# _C_MLT = +2*128 [128, 2*128] lateral masks transposed (j-part, i-free)


def _tab_offsets(nl):
    c_vzl = _C_VYL + nl
    c_mf = c_vzl + nl
    c_mlt = c_mf + 2 * V
    return c_vzl, c_mf, c_mlt, c_mlt + 2 * V

_PROGRAM_CACHE: dict = {}


class _GeometryFallback(Exception):
    pass


def _build_program(nl_ks: tuple):
    key = nl_ks
    if key in _PROGRAM_CACHE:
        return _PROGRAM_CACHE[key]

    nl = len(nl_ks)
    c_vzl, c_mf, c_mlt, tabs_w = _tab_offsets(nl)
    nc = bacc.Bacc("TRN2", target_bir_lowering=False, debug=False,
                   num_devices=N_CORES)
    tabs = nc.declare_dram_parameter("tabs", [128, tabs_w], BF16,
                                     isOutput=False)
    gt_p = nc.declare_dram_parameter("gt", [128, ZW * V], BF16, isOutput=False)
    out_p = nc.declare_dram_parameter("out_vec", [128, 8], F32, isOutput=True)

    with tile.TileContext(nc) as tc:
        with (
            tc.tile_pool(name="const", bufs=1) as constp,
            tc.tile_pool(name="vsb", bufs=3) as vsbp,
            tc.tile_pool(name="wsb", bufs=2) as wsbp,
            tc.tile_pool(name="bce", bufs=2) as bcep,
            tc.tile_pool(name="psV", bufs=2, space="PSUM") as psVp,
            tc.tile_pool(name="psCF", bufs=2, space="PSUM") as psCFp,
            tc.tile_pool(name="psW", bufs=2, space="PSUM") as psWp,
            tc.tile_pool(name="psC", bufs=2, space="PSUM") as psCp,
        ):
            tabs_sb = constp.tile([128, tabs_w], BF16)
            nc.sync.dma_start(tabs_sb[:], tabs.ap())
            gt_sb = constp.tile([128, ZW, V], BF16)
            nc.sync.dma_start(gt_sb[:], gt_p.ap())

            iota_i = constp.tile([128, V], I32)
            nc.gpsimd.iota(iota_i[:], pattern=[[1, V]], base=0,
                           channel_multiplier=0)
            iota_b = constp.tile([128, V], BF16)
            nc.vector.tensor_copy(iota_b[:], iota_i[:])

            def one_hot(name, val_ap, nk, nv):
                """AT[p, k, v] = (val[p, k] == v), bf16 {0,1}."""
                t = constp.tile([128, nk, nv], BF16, tag=name)
                nc.vector.tensor_tensor(
                    t[:],
                    iota_b[:, 0:nv].unsqueeze(1).broadcast_to([128, nk, nv]),
                    val_ap.unsqueeze(2).broadcast_to([128, nk, nv]),
                    ALU.is_equal)
                return t

            # lateral one-hots first: they gate the PE-dominant lateral pass
            ATzL = one_hot("ATzL", tabs_sb[:, c_vzl:c_vzl + nl], nl, ZW)
            ATyL = one_hot("ATyL", tabs_sb[:, _C_VYL:_C_VYL + nl], nl, V)
            ATxF = one_hot("ATxF", tabs_sb[:, _C_VXF:_C_VXF + ZW], ZW, V)
            ATyF = one_hot("ATyF", tabs_sb[:, _C_VYF:_C_VYF + ZW], ZW, V)

            def maskF(b):   # [i, j]
                return tabs_sb[:, c_mf + V * b:c_mf + V * (b + 1)]

            def maskLT(b):  # [j, i]
                return tabs_sb[:, c_mlt + V * b:c_mlt + V * (b + 1)]

            volF = constp.tile([128, 2, ZW, V], BF16)
            volL = constp.tile([128, 2, ZW, V], BF16)
            nc.gpsimd.memset(volL[:], 0.0)

            # ---- lateral: [y, z-band] columns of each non-empty x-slice,
            #      batched 8 slices per PSUM bank ----
            LB = 8
            for blk in range(0, nl, LB):
                ks = nl_ks[blk:blk + LB]
                psW = psWp.tile([128, LB, 2, ZW], F32)
                for slot, k in enumerate(ks):
                    pos = blk + slot
                    for b in range(2):
                        nc.tensor.matmul(psW[:, slot, b, :], lhsT=maskLT(b),
                                         rhs=ATzL[:, pos, :],
                                         start=True, stop=True)
                wsb = wsbp.tile([128, LB, 2, ZW], BF16, tag="w")
                nc.scalar.copy(wsb[:, 0:len(ks)], psW[:, 0:len(ks)])
                psC2 = psCp.tile([128, LB, 2, ZW], F32)
                for slot, k in enumerate(ks):
                    nc.tensor.matmul(psC2[:, slot], lhsT=ATyL[:, blk + slot, :],
                                     rhs=wsb[:, slot], start=True, stop=True)
                for slot, k in enumerate(ks):
                    nc.scalar.sign(volL[:, :, :, k], psC2[:, slot])

            # ---- frontal: full [y, x] slice per owned z ----
            for kk in range(ZW):
                psV = psVp.tile([128, 2, V], F32)
                for b in range(2):
                    nc.tensor.matmul(psV[:, b, :], lhsT=maskF(b),
                                     rhs=ATxF[:, kk, :], start=True, stop=True)
                vsb = vsbp.tile([128, 2, V], BF16, tag="v")
                nc.vector.tensor_copy(vsb[:], psV[:])
                psC = psCFp.tile([128, 2, V], F32)
                nc.tensor.matmul(psC[:], lhsT=ATyF[:, kk, :], rhs=vsb[:],
                                 start=True, stop=True)
                nc.scalar.sign(volF[:, :, kk, :], psC[:])

            # ---- BCE partial sums: out cols (3b+0, 3b+1, 3b+2) =
            #      sum(s), sum(s==2), sum(gt*s) for batch b ----
            out_sb = constp.tile([128, 8], F32)
            nc.gpsimd.memset(out_sb[:], 0.0)
            for b in range(2):
                s = bcep.tile([128, ZW, V], BF16, tag="s")
                nc.vector.scalar_tensor_tensor(
                    out=s[:], in0=volF[:, b], scalar=0.0, in1=volL[:, b],
                    op0=ALU.add, op1=ALU.add,
                    accum_out=out_sb[:, 3 * b:3 * b + 1])
                e2 = bcep.tile([128, ZW, V], BF16, tag="e2")
                nc.vector.tensor_scalar(
                    e2[:], s[:], 2.0, 0.0, ALU.is_equal, ALU.add,
                    accum_out=out_sb[:, 3 * b + 1:3 * b + 2])
                gs = bcep.tile([128, ZW, V], BF16, tag="gs")
                nc.vector.scalar_tensor_tensor(
                    out=gs[:], in0=gt_sb[:], scalar=1.0, in1=s[:],
                    op0=ALU.mult, op1=ALU.mult,
                    accum_out=out_sb[:, 3 * b + 2:3 * b + 3])
            nc.sync.dma_start(out_p.ap(), out_sb[:])

    nc.compile()
    _PROGRAM_CACHE[key] = nc
    return nc


def _host_prep(inputs):
    """Validate geometry and build per-core bf16 tables.

    Returns (in_maps, nl_ks).  Raises _GeometryFallback when the separability
    assumptions don't hold.
    """
    f32 = np.float32
    pf = np.asarray(inputs["pred_frontal"], dtype=f32)
    pl = np.asarray(inputs["pred_lateral"], dtype=f32)
    srcF = np.asarray(inputs["source_F"], dtype=np.float64)[0]
    tgtF = np.asarray(inputs["target_F"], dtype=np.float64)[0]
    srcL = np.asarray(inputs["source_L"], dtype=np.float64)[0]
    tgtL = np.asarray(inputs["target_L"], dtype=np.float64)[0]
    A_inv = np.asarray(inputs["A_inv"], dtype=np.float64)
    t_inv = np.asarray(inputs["t_inv"], dtype=np.float64)
    gt = np.asarray(inputs["vol_gt_3d"], dtype=f32)
    B = pf.shape[0]
    if B != 2 or gt.shape != (V, V, V) or pf.shape[2:] != (V, V):
        raise _GeometryFallback(f"unexpected shapes B={B}")
    if not np.array_equal(A_inv, np.diag(np.diag(A_inv))):
        raise _GeometryFallback("A_inv not diagonal")
    D = np.diag(A_inv)

    def view_tables(src, tgt, scan_ax, ax_i, ax_j):
        """Per-slice sample index + separable coordinate maps (f64)."""
        # target coordinate along scan axis must be globally constant;
        # along ax_i it may depend only on detector row i, ax_j only on j.
        c = tgt[0, 0, scan_ax]
        if not np.all(tgt[..., scan_ax] == c):
            raise _GeometryFallback("scan axis not constant")
        ti = tgt[:, 0, ax_i]
        if not np.all(tgt[..., ax_i] == ti[:, None]):
            raise _GeometryFallback("ax_i not separable")
        tj = tgt[0, :, ax_j]
        if not np.all(tgt[..., ax_j] == tj[None, :]):
            raise _GeometryFallback("ax_j not separable")

        beta = 2.5 * np.arange(S, dtype=np.float64) / (S - 1.0)
        zeta = (src[scan_ax] + (c - src[scan_ax]) * beta) * D[scan_ax] \
            + t_inv[scan_ax]
        ks = np.rint(zeta).astype(np.int64)
        margin = np.abs(np.abs(zeta - np.rint(zeta)) - 0.5).min()
        if margin < 5e-4:
            raise _GeometryFallback(f"scan margin {margin:.1e}")
        inb = (ks >= 0) & (ks < V)
        if len(np.unique(ks[inb])) != int(inb.sum()):
            raise _GeometryFallback("multiple samples per slice")
        s_for_k = np.full(V, -1, np.int64)
        s_for_k[ks[inb]] = np.arange(S)[inb]

        p = np.arange(V, dtype=np.float64)

        def cmap(tvals, axis):
            """[p, k] voxel coordinate map with POISON for invalid entries."""
            out = np.full((V, V), POISON, dtype=np.float64)
            for k in range(V):
                sk = s_for_k[k]
                if sk < 0:
                    continue
                a = beta[sk]
                w = (src[axis] + (tvals - src[axis]) * a) * D[axis] \
                    + t_inv[axis]
                m = np.abs(np.abs(w - np.rint(w)) - 0.5).min()
                if m < 5e-4:
                    raise _GeometryFallback(f"transverse margin {m:.1e}")
                r = np.rint(w)
                r[(r < 0) | (r >= V)] = POISON
                out[:, k] = r
            return out

        return s_for_k, cmap(ti, ax_i), cmap(tj, ax_j)

    # frontal: scan z(2), i -> vol axis 0 (x), j -> vol axis 1 (y)
    sfF, mapxF, mapyF = view_tables(srcF, tgtF, 2, 0, 1)
    # lateral: scan x(0), i -> vol axis 1 (y), j -> vol axis 2 (z)
    sfL, mapyL, mapzL = view_tables(srcL, tgtL, 0, 1, 2)

    nl_ks = tuple(int(k) for k in range(V) if sfL[k] >= 0)
    if not nl_ks:
        nl_ks = (0,)  # degenerate but keeps the program shape valid

    bf16 = ml_dtypes.bfloat16
    nl = len(nl_ks)
    c_vzl, c_mf, c_mlt, tabs_w = _tab_offsets(nl)
    klist = np.array(nl_ks, dtype=np.int64)
    maskF = (pf[:, 0] > 0.5)                       # [b, i, j]
    maskLT = (pl[:, 0] > 0.5).transpose(0, 2, 1)    # [b, j, i]
    gtzyx = np.ascontiguousarray(gt.transpose(1, 2, 0))  # [y][z][x]

    in_maps = []
    for cidx in range(N_CORES):
        z0 = ZW * cidx
        tabs = np.full((128, tabs_w), POISON, dtype=np.float64)
        tabs[:, _C_VXF:_C_VXF + ZW] = mapxF[:, z0:z0 + ZW]
        tabs[:, _C_VYF:_C_VYF + ZW] = mapyF[:, z0:z0 + ZW]
        tabs[:, _C_VYL:_C_VYL + nl] = mapyL[:, klist]
        vz = mapzL[:, klist].copy()
        ok = vz != POISON
        vz[ok] = vz[ok] - z0
        tabs[:, c_vzl:c_vzl + nl] = vz
        tabs[:, c_mf:c_mf + V] = maskF[0]
        tabs[:, c_mf + V:c_mf + 2 * V] = maskF[1]
        tabs[:, c_mlt:c_mlt + V] = maskLT[0]
        tabs[:, c_mlt + V:c_mlt + 2 * V] = maskLT[1]
        gshard = gtzyx[:, z0:z0 + ZW, :].reshape(128, ZW * V)
        in_maps.append({"tabs": tabs.astype(bf16),
                        "gt": gshard.astype(bf16)})
    return in_maps, nl_ks


def _combine(results) -> np.ndarray:
    """Host-side reduction of the 8 per-core [128, 8] partial-sum tensors."""
    acc = np.zeros(8, dtype=np.float64)
    for r in results:
        acc += np.asarray(r["out_vec"], dtype=np.float64).sum(axis=0)
    total = 0.0
    for b in range(2):
        ss, se2, sgs = acc[3 * b], acc[3 * b + 1], acc[3 * b + 2]
        total += Q0 * (V ** 3) + (Q1 + Q2) * ss + 2.0 * Q2 * se2 + sgs
    return np.float32(-total / (2.0 * V ** 3))


def _reference_fallback(inputs):
    """Faithful f32 numpy replica of the jax reference (safety net)."""
    f32 = np.float32
    pf = np.asarray(inputs["pred_frontal"], dtype=f32)
    pl = np.asarray(inputs["pred_lateral"], dtype=f32)
    srcF = np.asarray(inputs["source_F"], dtype=f32)[0]
    tgtF = np.asarray(inputs["target_F"], dtype=f32)[0]
    srcL = np.asarray(inputs["source_L"], dtype=f32)[0]
    tgtL = np.asarray(inputs["target_L"], dtype=f32)[0]
    A_inv = np.asarray(inputs["A_inv"], dtype=f32)
    t_inv = np.asarray(inputs["t_inv"], dtype=f32)
    gt = np.asarray(inputs["vol_gt_3d"], dtype=f32)

    def backproject(mask2d, src, tgt):
        active = (mask2d > 0.5).reshape(-1)
        det = tgt.reshape(-1, 3).astype(f32)
        rd = (det - src[None, :]).astype(f32)
        rl = np.sqrt((rd * rd).sum(1, dtype=f32)).astype(f32)[:, None]
        rdn = (rd / (rl + f32(1e-8))).astype(f32)
        tv = (np.arange(S, dtype=f32) * (f32(1.0) / f32(S - 1)))
        ts = (tv[None, :, None] * (rl[:, None, :] * f32(2.5))).astype(f32)
        world = (src[None, None, :] + rdn[:, None, :] * ts).astype(f32)
        vox_f = (world @ A_inv.T + t_inv).astype(f32)
        vox = np.rint(vox_f).astype(np.int64)
        ok = (active[:, None]
              & (vox[..., 0] >= 0) & (vox[..., 0] < V)
              & (vox[..., 1] >= 0) & (vox[..., 1] < V)
              & (vox[..., 2] >= 0) & (vox[..., 2] < V))
        vi = np.clip(vox, 0, V - 1)
        vol = np.zeros((V, V, V), dtype=f32)
        flat = (vi[..., 0] * V + vi[..., 1]) * V + vi[..., 2]
        vol.reshape(-1)[flat[ok]] = 1.0
        return vol

    total = 0.0
    B = pf.shape[0]
    for b in range(B):
        vF = backproject(pf[b, 0], srcF, tgtF)
        vL = backproject(pl[b, 0], srcL, tgtL)
        sv = (vF + vL).astype(np.float64)
        p = 1.0 / (1.0 + np.exp(-sv))
        total += -(gt * np.log(p) + (1.0 - gt) * np.log1p(-p)).mean()
    return np.float32(total / B)


def kernel(**inputs) -> np.ndarray:
    try:
        in_maps, nl_ks = _host_prep(inputs)
    except _GeometryFallback:
        return _reference_fallback(inputs)
    nc = _build_program(nl_ks)
    res = run_bass_kernel_spmd(nc, in_maps, list(range(N_CORES)))
    return _combine(res.results)


# revision 15
# speedup vs baseline: 4.5896x; 1.2444x over previous
"""Trainium2 Bass kernel: Backprojection3DConsistencyLoss (8-core SPMD).

Contract: kernel(**inputs) takes the FULL unsharded inputs of the reference
(pred_frontal/pred_lateral [2,1,128,128] f32, source/target geometry, the
ground-truth volume [128,128,128] f32, A_inv [3,3], t_inv [3]) and returns the
FULL scalar loss, computing the heavy work on 8 NeuronCores.

Algorithm (separable per-slice reconstruction; no collectives):
  For this module's geometry the detector plane is constant along the scan
  axis, so every ray shares the same scan-axis sample sequence
  z_s = src_z + dz*(2.5 s/511) (the ray-length normalization cancels exactly).
  Slice k therefore has a unique shared sample s_k (or none), and within the
  slice the hit voxel is a separable affine map of the detector indices:
  y = rint(sy + (j-sy)*a_k), x = rint(sx + (i-sx)*a_k) with a_k = 2.5 s_k/511.
  With one-hot matrices A_k[p, v] = [map_k(p) == v] the 0/1 slice image is
  sign(A_k^T . mask^T-ish . A_k) — two small matmuls on the TensorEngine.

  Sharding: core c owns z-slices [16c, 16c+16) of all four volumes.
  Frontal volumes are built slice-by-slice (z = slice axis); lateral volumes
  (x = slice axis) are built as the 16-column z-band of every x-slice, via a
  skinny pair of matmuls per non-empty x-slice.  All per-core variation is
  carried by host-computed bf16 lookup tables, so one SPMD program serves all
  8 cores and no ReduceScatter is needed.  Each core evaluates its BCE shard
  with the exact quadratic q0 + q1*s + q2*s^2 + gt*s (s in {0,1,2}) reduced
  on-device to a few per-partition sums; the host combines them.

If the input geometry violates the separability assumptions (it holds for
this module's detector geometry; margins are checked in f64), a faithful f32
numpy fallback computes the same result on host.
"""

import math
import sys

import numpy as np

for _p in ("/opt/trn_rl_repo",):
    if _p not in sys.path:
        sys.path.insert(0, _p)

import ml_dtypes  # noqa: E402

import concourse.bacc as bacc  # noqa: E402
import concourse.mybir as mybir  # noqa: E402
import concourse.tile as tile  # noqa: E402
from concourse.bass_utils import run_bass_kernel_spmd  # noqa: E402

N_CORES = 8
V = 128          # volume side
S = 512          # samples per ray
ZW = V // N_CORES  # z-slices per core (16)
POISON = 200.0   # one-hot compare value that never matches a voxel coordinate
F32 = mybir.dt.float32
BF16 = mybir.dt.bfloat16
I32 = mybir.dt.int32
ALU = mybir.AluOpType

# BCE quadratic: cell loss = q0 + q1*s + q2*s^2 + gt*s, exact for s in {0,1,2}
_B0 = math.log(0.5)
_B1 = -math.log1p(math.e)
_B2 = -2.0 - math.log1p(math.exp(-2.0))
Q0 = _B0
Q1 = (-3.0 * _B0 + 4.0 * _B1 - _B2) / 2.0
Q2 = (_B0 - 2.0 * _B1 + _B2) / 2.0

# packed bf16 table layout (columns of the [128, W] "tabs" input); lateral
# tables carry only the NL non-empty x-slices, in nl_ks order
_C_VXF = 0      # [128, 16]  frontal x map for the core's 16 z-slices
_C_VYF = 16     # [128, 16]  frontal y map
_C_VYL = 32     # [128, NL]  lateral y map
# _C_VZL = 32+NL  [128, NL]  lateral z map, shifted by -16c
# _C_MF  = 32+2NL [128, 2*128] frontal masks (i-partition, j-free)
# _C_MLT = +2*128 [128, 2*128] lateral masks transposed (j-part, i-free)


def _tab_offsets(nl):
    c_vzl = _C_VYL + nl
    c_mf = c_vzl + nl
    c_mlt = c_mf + 2 * V
    return c_vzl, c_mf, c_mlt, c_mlt + 2 * V

_PROGRAM_CACHE: dict = {}


class _GeometryFallback(Exception):
    pass


def _build_program(nl_ks: tuple):
    key = nl_ks
    if key in _PROGRAM_CACHE:
        return _PROGRAM_CACHE[key]

    nl = len(nl_ks)
    c_vzl, c_mf, c_mlt, tabs_w = _tab_offsets(nl)
    nc = bacc.Bacc("TRN2", target_bir_lowering=False, debug=False,
                   num_devices=N_CORES)
    tabs = nc.declare_dram_parameter("tabs", [128, tabs_w], BF16,
                                     isOutput=False)
    gt_p = nc.declare_dram_parameter("gt", [128, ZW * V], BF16, isOutput=False)
    out_p = nc.declare_dram_parameter("out_vec", [128, 8], F32, isOutput=True)

    with tile.TileContext(nc) as tc:
        with (
            tc.tile_pool(name="const", bufs=1) as constp,
            tc.tile_pool(name="vsb", bufs=3) as vsbp,
            tc.tile_pool(name="wsb", bufs=2) as wsbp,
            tc.tile_pool(name="bce", bufs=2) as bcep,
            tc.tile_pool(name="psV", bufs=2, space="PSUM") as psVp,
            tc.tile_pool(name="psCF", bufs=2, space="PSUM") as psCFp,
            tc.tile_pool(name="psW", bufs=2, space="PSUM") as psWp,
            tc.tile_pool(name="psC", bufs=2, space="PSUM") as psCp,
        ):
            tabs_sb = constp.tile([128, tabs_w], BF16)
            nc.sync.dma_start(tabs_sb[:], tabs.ap())
            gt_sb = constp.tile([128, ZW, V], BF16)
            nc.sync.dma_start(gt_sb[:], gt_p.ap())

            iota_i = constp.tile([128, V], I32)
            nc.gpsimd.iota(iota_i[:], pattern=[[1, V]], base=0,
                           channel_multiplier=0)
            iota_b = constp.tile([128, V], BF16)
            nc.vector.tensor_copy(iota_b[:], iota_i[:])

            def one_hot(name, val_ap, nk, nv):
                """AT[p, k, v] = (val[p, k] == v), bf16 {0,1}."""
                t = constp.tile([128, nk, nv], BF16, tag=name)
                nc.vector.tensor_tensor(
                    t[:],
                    iota_b[:, 0:nv].unsqueeze(1).broadcast_to([128, nk, nv]),
                    val_ap.unsqueeze(2).broadcast_to([128, nk, nv]),
                    ALU.is_equal)
                return t

            # lateral one-hots first: they gate the PE-dominant lateral pass
            ATzL = one_hot("ATzL", tabs_sb[:, c_vzl:c_vzl + nl], nl, ZW)
            ATyL = one_hot("ATyL", tabs_sb[:, _C_VYL:_C_VYL + nl], nl, V)
            ATxF = one_hot("ATxF", tabs_sb[:, _C_VXF:_C_VXF + ZW], ZW, V)
            ATyF = one_hot("ATyF", tabs_sb[:, _C_VYF:_C_VYF + ZW], ZW, V)

            def maskF(b):   # [i, j]
                return tabs_sb[:, c_mf + V * b:c_mf + V * (b + 1)]

            def maskLT(b):  # [j, i]
                return tabs_sb[:, c_mlt + V * b:c_mlt + V * (b + 1)]

            volF = constp.tile([128, 2, ZW, V], BF16)
            volL = constp.tile([128, 2, ZW, V], BF16)
            nc.gpsimd.memset(volL[:], 0.0)

            # ---- lateral: [y, z-band] columns of each non-empty x-slice,
            #      batched 8 slices per PSUM bank; mm1 is one wide matmul
            #      per (block, batch) since its weights (the mask) are fixed
            LB = 8
            for blk in range(0, nl, LB):
                ks = nl_ks[blk:blk + LB]
                nb = len(ks)
                psW = psWp.tile([128, 2, LB, ZW], F32)
                for b in range(2):
                    nc.tensor.matmul(psW[:, b, 0:nb, :], lhsT=maskLT(b),
                                     rhs=ATzL[:, blk:blk + nb, :],
                                     start=True, stop=True)
                wsb = wsbp.tile([128, 2, LB, ZW], BF16, tag="w")
                nc.scalar.copy(wsb[:, :, 0:nb, :], psW[:, :, 0:nb, :])
                psC2 = psCp.tile([128, LB, 2, ZW], F32)
                for slot, k in enumerate(ks):
                    nc.tensor.matmul(psC2[:, slot], lhsT=ATyL[:, blk + slot, :],
                                     rhs=wsb[:, :, slot, :],
                                     start=True, stop=True)
                for slot, k in enumerate(ks):
                    nc.scalar.sign(volL[:, :, :, k], psC2[:, slot])

            # ---- frontal: full [y, x] slice per owned z, mm1 four slices
            #      wide per batch ----
            FB = 2
            for k0 in range(0, ZW, FB):
                psV = psVp.tile([128, 2, FB, V], F32)
                for b in range(2):
                    nc.tensor.matmul(psV[:, b], lhsT=maskF(b),
                                     rhs=ATxF[:, k0:k0 + FB, :],
                                     start=True, stop=True)
                vsb = vsbp.tile([128, 2, FB, V], BF16, tag="v")
                nc.vector.tensor_copy(vsb[:], psV[:])
                for kl in range(FB):
                    kk = k0 + kl
                    psC = psCFp.tile([128, 2, V], F32)
                    nc.tensor.matmul(psC[:], lhsT=ATyF[:, kk, :],
                                     rhs=vsb[:, :, kl, :],
                                     start=True, stop=True)
                    nc.scalar.sign(volF[:, :, kk, :], psC[:])

            # ---- BCE partial sums: out cols (3b+0, 3b+1, 3b+2) =
            #      sum(s), sum(s==2), sum(gt*s) for batch b ----
            out_sb = constp.tile([128, 8], F32)
            nc.gpsimd.memset(out_sb[:], 0.0)
            for b in range(2):
                s = bcep.tile([128, ZW, V], BF16, tag="s")
                nc.vector.scalar_tensor_tensor(
                    out=s[:], in0=volF[:, b], scalar=0.0, in1=volL[:, b],
                    op0=ALU.add, op1=ALU.add,
                    accum_out=out_sb[:, 3 * b:3 * b + 1])
                e2 = bcep.tile([128, ZW, V], BF16, tag="e2")
                nc.vector.tensor_scalar(
                    e2[:], s[:], 2.0, 0.0, ALU.is_equal, ALU.add,
                    accum_out=out_sb[:, 3 * b + 1:3 * b + 2])
                gs = bcep.tile([128, ZW, V], BF16, tag="gs")
                nc.vector.scalar_tensor_tensor(
                    out=gs[:], in0=gt_sb[:], scalar=1.0, in1=s[:],
                    op0=ALU.mult, op1=ALU.mult,
                    accum_out=out_sb[:, 3 * b + 2:3 * b + 3])
            nc.sync.dma_start(out_p.ap(), out_sb[:])

    nc.compile()
    _PROGRAM_CACHE[key] = nc
    return nc


def _host_prep(inputs):
    """Validate geometry and build per-core bf16 tables.

    Returns (in_maps, nl_ks).  Raises _GeometryFallback when the separability
    assumptions don't hold.
    """
    f32 = np.float32
    pf = np.asarray(inputs["pred_frontal"], dtype=f32)
    pl = np.asarray(inputs["pred_lateral"], dtype=f32)
    srcF = np.asarray(inputs["source_F"], dtype=np.float64)[0]
    tgtF = np.asarray(inputs["target_F"], dtype=np.float64)[0]
    srcL = np.asarray(inputs["source_L"], dtype=np.float64)[0]
    tgtL = np.asarray(inputs["target_L"], dtype=np.float64)[0]
    A_inv = np.asarray(inputs["A_inv"], dtype=np.float64)
    t_inv = np.asarray(inputs["t_inv"], dtype=np.float64)
    gt = np.asarray(inputs["vol_gt_3d"], dtype=f32)
    B = pf.shape[0]
    if B != 2 or gt.shape != (V, V, V) or pf.shape[2:] != (V, V):
        raise _GeometryFallback(f"unexpected shapes B={B}")
    if not np.array_equal(A_inv, np.diag(np.diag(A_inv))):
        raise _GeometryFallback("A_inv not diagonal")
    D = np.diag(A_inv)

    def view_tables(src, tgt, scan_ax, ax_i, ax_j):
        """Per-slice sample index + separable coordinate maps (f64)."""
        # target coordinate along scan axis must be globally constant;
        # along ax_i it may depend only on detector row i, ax_j only on j.
        c = tgt[0, 0, scan_ax]
        if not np.all(tgt[..., scan_ax] == c):
            raise _GeometryFallback("scan axis not constant")
        ti = tgt[:, 0, ax_i]
        if not np.all(tgt[..., ax_i] == ti[:, None]):
            raise _GeometryFallback("ax_i not separable")
        tj = tgt[0, :, ax_j]
        if not np.all(tgt[..., ax_j] == tj[None, :]):
            raise _GeometryFallback("ax_j not separable")

        beta = 2.5 * np.arange(S, dtype=np.float64) / (S - 1.0)
        zeta = (src[scan_ax] + (c - src[scan_ax]) * beta) * D[scan_ax] \
            + t_inv[scan_ax]
        ks = np.rint(zeta).astype(np.int64)
        margin = np.abs(np.abs(zeta - np.rint(zeta)) - 0.5).min()
        if margin < 5e-4:
            raise _GeometryFallback(f"scan margin {margin:.1e}")
        inb = (ks >= 0) & (ks < V)
        if len(np.unique(ks[inb])) != int(inb.sum()):
            raise _GeometryFallback("multiple samples per slice")
        s_for_k = np.full(V, -1, np.int64)
        s_for_k[ks[inb]] = np.arange(S)[inb]

        p = np.arange(V, dtype=np.float64)

        def cmap(tvals, axis):
            """[p, k] voxel coordinate map with POISON for invalid entries."""
            out = np.full((V, V), POISON, dtype=np.float64)
            for k in range(V):
                sk = s_for_k[k]
                if sk < 0:
                    continue
                a = beta[sk]
                w = (src[axis] + (tvals - src[axis]) * a) * D[axis] \
                    + t_inv[axis]
                m = np.abs(np.abs(w - np.rint(w)) - 0.5).min()
                if m < 5e-4:
                    raise _GeometryFallback(f"transverse margin {m:.1e}")
                r = np.rint(w)
                r[(r < 0) | (r >= V)] = POISON
                out[:, k] = r
            return out

        return s_for_k, cmap(ti, ax_i), cmap(tj, ax_j)

    # frontal: scan z(2), i -> vol axis 0 (x), j -> vol axis 1 (y)
    sfF, mapxF, mapyF = view_tables(srcF, tgtF, 2, 0, 1)
    # lateral: scan x(0), i -> vol axis 1 (y), j -> vol axis 2 (z)
    sfL, mapyL, mapzL = view_tables(srcL, tgtL, 0, 1, 2)

    nl_ks = tuple(int(k) for k in range(V) if sfL[k] >= 0)
    if not nl_ks:
        nl_ks = (0,)  # degenerate but keeps the program shape valid

    bf16 = ml_dtypes.bfloat16
    nl = len(nl_ks)
    c_vzl, c_mf, c_mlt, tabs_w = _tab_offsets(nl)
    klist = np.array(nl_ks, dtype=np.int64)
    maskF = (pf[:, 0] > 0.5)                       # [b, i, j]
    maskLT = (pl[:, 0] > 0.5).transpose(0, 2, 1)    # [b, j, i]
    gtzyx = np.ascontiguousarray(gt.transpose(1, 2, 0))  # [y][z][x]

    in_maps = []
    for cidx in range(N_CORES):
        z0 = ZW * cidx
        tabs = np.full((128, tabs_w), POISON, dtype=np.float64)
        tabs[:, _C_VXF:_C_VXF + ZW] = mapxF[:, z0:z0 + ZW]
        tabs[:, _C_VYF:_C_VYF + ZW] = mapyF[:, z0:z0 + ZW]
        tabs[:, _C_VYL:_C_VYL + nl] = mapyL[:, klist]
        vz = mapzL[:, klist].copy()
        ok = vz != POISON
        vz[ok] = vz[ok] - z0
        tabs[:, c_vzl:c_vzl + nl] = vz
        tabs[:, c_mf:c_mf + V] = maskF[0]
        tabs[:, c_mf + V:c_mf + 2 * V] = maskF[1]
        tabs[:, c_mlt:c_mlt + V] = maskLT[0]
        tabs[:, c_mlt + V:c_mlt + 2 * V] = maskLT[1]
        gshard = gtzyx[:, z0:z0 + ZW, :].reshape(128, ZW * V)
        in_maps.append({"tabs": tabs.astype(bf16),
                        "gt": gshard.astype(bf16)})
    return in_maps, nl_ks


def _combine(results) -> np.ndarray:
    """Host-side reduction of the 8 per-core [128, 8] partial-sum tensors."""
    acc = np.zeros(8, dtype=np.float64)
    for r in results:
        acc += np.asarray(r["out_vec"], dtype=np.float64).sum(axis=0)
    total = 0.0
    for b in range(2):
        ss, se2, sgs = acc[3 * b], acc[3 * b + 1], acc[3 * b + 2]
        total += Q0 * (V ** 3) + (Q1 + Q2) * ss + 2.0 * Q2 * se2 + sgs
    return np.float32(-total / (2.0 * V ** 3))


def _reference_fallback(inputs):
    """Faithful f32 numpy replica of the jax reference (safety net)."""
    f32 = np.float32
    pf = np.asarray(inputs["pred_frontal"], dtype=f32)
    pl = np.asarray(inputs["pred_lateral"], dtype=f32)
    srcF = np.asarray(inputs["source_F"], dtype=f32)[0]
    tgtF = np.asarray(inputs["target_F"], dtype=f32)[0]
    srcL = np.asarray(inputs["source_L"], dtype=f32)[0]
    tgtL = np.asarray(inputs["target_L"], dtype=f32)[0]
    A_inv = np.asarray(inputs["A_inv"], dtype=f32)
    t_inv = np.asarray(inputs["t_inv"], dtype=f32)
    gt = np.asarray(inputs["vol_gt_3d"], dtype=f32)

    def backproject(mask2d, src, tgt):
        active = (mask2d > 0.5).reshape(-1)
        det = tgt.reshape(-1, 3).astype(f32)
        rd = (det - src[None, :]).astype(f32)
        rl = np.sqrt((rd * rd).sum(1, dtype=f32)).astype(f32)[:, None]
        rdn = (rd / (rl + f32(1e-8))).astype(f32)
        tv = (np.arange(S, dtype=f32) * (f32(1.0) / f32(S - 1)))
        ts = (tv[None, :, None] * (rl[:, None, :] * f32(2.5))).astype(f32)
        world = (src[None, None, :] + rdn[:, None, :] * ts).astype(f32)
        vox_f = (world @ A_inv.T + t_inv).astype(f32)
        vox = np.rint(vox_f).astype(np.int64)
        ok = (active[:, None]
              & (vox[..., 0] >= 0) & (vox[..., 0] < V)
              & (vox[..., 1] >= 0) & (vox[..., 1] < V)
              & (vox[..., 2] >= 0) & (vox[..., 2] < V))
        vi = np.clip(vox, 0, V - 1)
        vol = np.zeros((V, V, V), dtype=f32)
        flat = (vi[..., 0] * V + vi[..., 1]) * V + vi[..., 2]
        vol.reshape(-1)[flat[ok]] = 1.0
        return vol

    total = 0.0
    B = pf.shape[0]
    for b in range(B):
        vF = backproject(pf[b, 0], srcF, tgtF)
        vL = backproject(pl[b, 0], srcL, tgtL)
        sv = (vF + vL).astype(np.float64)
        p = 1.0 / (1.0 + np.exp(-sv))
        total += -(gt * np.log(p) + (1.0 - gt) * np.log1p(-p)).mean()
    return np.float32(total / B)


def kernel(**inputs) -> np.ndarray:
    try:
        in_maps, nl_ks = _host_prep(inputs)
    except _GeometryFallback:
        return _reference_fallback(inputs)
    nc = _build_program(nl_ks)
    res = run_bass_kernel_spmd(nc, in_maps, list(range(N_CORES)))
    return _combine(res.results)
